# revision 4
# baseline (speedup 1.0000x reference)
"""Trainium2 Bass kernel for nn_DiagonalTraining (ragged per-anti-diagonal linear).

Math (reference): for each batch image x[b] (SxS) and each anti-diagonal
i (elements x[b, r, i-r], r=0..i), apply a per-diagonal linear layer:
  out[b,i,q] = sum_{r<=i} x[b,r,i-r] * W[i,q,r] + bias[i,q]   (q <= i)
and scatter back: y[b,q,i-q] = out[b,i,q]; positions with r+c >= S keep x.

Distribution: diagonal i -> core i%8, slot j=i//8 (64 slots per core,
balanced by construction). Host packs, per (core, slot), an augmented
matrix whose rows are the contraction axis r:
  [ D^T | V ]  with D^T[r,b]=x[b,r,i-r], V[r,q]=W[i,q,r]  (r,q < ni=i+1)
plus one extra row [ 1...1 | bias ] so the bias-add is a free extra
contraction row, zero-padded to a core-independent size NJ=8*(j+1)
(>= ni for every core) so the SPMD program is identical on all cores.

Device: per slot, ONE DMA loads the slot in partition-major layout
(rows r=c*h+p stored at partition p, segment c), giving h (<=128) large
descriptors per slot instead of one per row. The tensor engine
accumulates  psum[32, NJ] += chunk_c[:, :32].T @ chunk_c[:, 32:]
over the s chunks, DVE copies psum to SBUF, and the result DMAs to a
packed output blob. Host scatters the blobs back into a copy of x.

Only the live (lower-triangular) part of W is shipped/read (~25 MB/core
vs 512 MB full W) — the kernel is HBM-bound on exactly those bytes.
Loads alternate between the two HWDGE rings (sync + scalar engines);
stores go through SWDGE (gpsimd) so descriptor generation is parallel.
"""

import sys

for _p in ("/opt/trn_rl_repo", "/opt/pypackages"):
    if _p not in sys.path:
        sys.path.append(_p)

import numpy as np

import concourse.bass as bass  # noqa: F401
import concourse.tile as tile
from concourse import bacc, mybir
from concourse.bass_utils import run_bass_kernel_spmd

B = 32          # batch
S = 512         # seq len / number of diagonals
N_CORES = 8
N_SLOTS = S // N_CORES  # 64 slots per core
DCOL = B        # width of the D^T block (batch on matmul M axis)

# tuning knobs (test.py may override before first kernel() call)
KCFG = {
    "compute": "f32",   # "f32" | "f32r"
    "order": "desc",    # slot issue order: largest-first
    "in_bufs": 4,
    "psum_bufs": 8,
    "out_bufs": 4,
}


def _slot_geom(j):
    """(NJ, wd, rows, s, h): chunk count s, chunk height h (=partitions)."""
    NJ = 8 * (j + 1)
    wd = DCOL + NJ
    rows = NJ + 1                       # diag rows + bias row
    s = -(-rows // 128)                 # ceil
    h = -(-rows // s)
    return NJ, wd, rows, s, h


_SLOT_OFF = []
_OUT_OFF = []
_off = 0
_ooff = 0
for _j in range(N_SLOTS):
    _NJ, _wd, _rows, _s, _h = _slot_geom(_j)
    _SLOT_OFF.append(_off)
    _OUT_OFF.append(_ooff)
    _off += _h * _s * _wd
    _ooff += B * _NJ
BLOB_ELEMS = _off
OUT_ELEMS = _ooff

_compiled_nc = None


def _build_program():
    global _compiled_nc
    if _compiled_nc is not None:
        return _compiled_nc

    from contextlib import ExitStack

    nc = bacc.Bacc("TRN2", target_bir_lowering=False, debug=False)
    f32 = mybir.dt.float32
    mm_dt = {"f32": f32, "f32r": mybir.dt.float32r}[KCFG["compute"]]
    blob = nc.dram_tensor("blob", [BLOB_ELEMS], f32, kind="ExternalInput").ap()
    outb = nc.dram_tensor("outblob", [OUT_ELEMS], f32, kind="ExternalOutput").ap()

    order = range(N_SLOTS - 1, -1, -1) if KCFG["order"] == "desc" else range(N_SLOTS)

    with tile.TileContext(nc) as tc, ExitStack() as ctx:
        in_pool = ctx.enter_context(tc.tile_pool(name="in", bufs=KCFG["in_bufs"]))
        out_pool = ctx.enter_context(tc.tile_pool(name="out", bufs=KCFG["out_bufs"]))
        psum_pool = ctx.enter_context(
            tc.tile_pool(name="psum", bufs=KCFG["psum_bufs"], space="PSUM")
        )

        for idx, j in enumerate(order):
            NJ, wd, rows, s, h = _slot_geom(j)
            base = _SLOT_OFF[j]

            t = in_pool.tile([h, s * wd], f32)
            src = blob[base : base + h * s * wd].rearrange(
                "(p f) -> p f", p=h, f=s * wd
            )
            ldeng = nc.sync if idx % 2 == 0 else nc.scalar
            ldeng.dma_start(t[:], src)

            psum_t = psum_pool.tile([B, NJ], f32)
            tv = t[:].bitcast(mm_dt)
            for c in range(s):
                nc.tensor.matmul(
                    psum_t[:],
                    tv[:, c * wd : c * wd + DCOL],
                    tv[:, c * wd + DCOL : (c + 1) * wd],
                    start=(c == 0),
                    stop=(c == s - 1),
                )
            out_t = out_pool.tile([B, NJ], f32)
            nc.vector.tensor_copy(out_t[:], psum_t[:])
            dst = outb[_OUT_OFF[j] : _OUT_OFF[j] + B * NJ].rearrange(
                "(p w) -> p w", p=B, w=NJ
            )
            nc.gpsimd.dma_start(dst, out_t[:])

    nc.compile()
    _compiled_nc = nc
    return nc


def _pack_core(k, x, W, bias):
    blob = np.zeros(BLOB_ELEMS, np.float32)
    for j in range(N_SLOTS):
        i = N_CORES * j + k
        ni = i + 1
        NJ, wd, rows, s, h = _slot_geom(j)
        M = np.zeros((h * s, wd), np.float32)
        r = np.arange(ni)
        M[:ni, :DCOL] = x[:, r, i - r].T               # D^T[r, b]
        M[NJ, :DCOL] = 1.0                             # ones row -> bias add
        M[:ni, DCOL : DCOL + ni] = W[i, :ni, :ni].T    # V[r, q]
        M[NJ, DCOL : DCOL + ni] = bias[i, :ni]
        # partition-major: row c*h+p -> partition p, segment c
        pm = M.reshape(s, h, wd).transpose(1, 0, 2)
        blob[_SLOT_OFF[j] : _SLOT_OFF[j] + h * s * wd] = pm.reshape(-1)
    return blob


def kernel(x, W, b):
    x = np.asarray(x, np.float32)
    W = np.asarray(W, np.float32)
    b = np.asarray(b, np.float32)

    nc = _build_program()
    in_maps = [{"blob": _pack_core(k, x, W, b)} for k in range(N_CORES)]
    res = run_bass_kernel_spmd(nc, in_maps, list(range(N_CORES)))

    y = x.copy()
    for k in range(N_CORES):
        ob = res.results[k]["outblob"]
        for j in range(N_SLOTS):
            i = N_CORES * j + k
            ni = i + 1
            NJ = 8 * (j + 1)
            o = ob[_OUT_OFF[j] : _OUT_OFF[j] + B * NJ].reshape(B, NJ)
            q = np.arange(ni)
            y[:, q, i - q] = o[:, :ni]
    return y


# revision 5
# speedup vs baseline: 2.1610x; 2.1610x over previous
"""Trainium2 Bass kernel for nn_DiagonalTraining (ragged per-anti-diagonal linear).

Math (reference): for each batch image x[b] (SxS) and each anti-diagonal
i (elements x[b, r, i-r], r=0..i), apply a per-diagonal linear layer:
  out[b,i,q] = sum_{r<=i} x[b,r,i-r] * W[i,q,r] + bias[i,q]   (q <= i)
and scatter back: y[b,q,i-q] = out[b,i,q]; positions with r+c >= S keep x.

Distribution: diagonal i -> core i%8, slot j=i//8 (64 slots per core,
balanced by construction). Host packs, per (core, slot), an augmented
matrix whose rows are the contraction axis r:
  [ D^T | V ]  with D^T[r,b]=x[b,r,i-r], V[r,q]=W[i,q,r]  (r,q < ni=i+1)
zero-padded to a core-independent size NJ=8*(j+1) (>= ni for every
core) so the SPMD program is identical on all cores. The per-diagonal
bias is added on the host while scattering results back (elementwise,
~0.05% of the FLOPs; the whole einsum runs on device).

Device: per slot, ONE SWDGE DMA loads the slot in partition-major
layout (row c*h+p stored at partition p, segment c), giving h (<=128)
large descriptors balanced across all 16 SDMA engines. The tensor
engine accumulates  psum[32, NJ] += chunk_c[:, :32].T @ chunk_c[:, 32:]
over the s chunks (float32r operands: full 32-bit data, 1 cycle/column
at N>=256), DVE copies psum into a quarter-group SBUF accumulator, and
one DMA per quarter stores the packed outputs. Host scatters them back
into a copy of x.

Only the live (lower-triangular) part of W is shipped/read (~25 MB/core
vs 512 MB full W) — the kernel is HBM-bound on exactly those bytes.
"""

import sys

for _p in ("/opt/trn_rl_repo", "/opt/pypackages"):
    if _p not in sys.path:
        sys.path.append(_p)

import numpy as np

import concourse.bass as bass  # noqa: F401
import concourse.tile as tile
from concourse import bacc, mybir
from concourse.bass_utils import run_bass_kernel_spmd

B = 32          # batch
S = 512         # seq len / number of diagonals
N_CORES = 8
N_SLOTS = S // N_CORES  # 64 slots per core
DCOL = B        # width of the D^T block (batch on matmul M axis)
N_STORE_GROUPS = 4

# tuning knobs (test.py may override before first kernel() call)
KCFG = {
    "compute": "f32r",  # "f32" | "f32r"
    "in_bufs": 4,
    "psum_bufs": 8,
}


def _slot_geom(j):
    """(NJ, wd, s, h): chunk count s, chunk height h (=partitions)."""
    NJ = 8 * (j + 1)
    wd = DCOL + NJ
    s = -(-NJ // 128)                   # ceil
    h = -(-NJ // s)
    return NJ, wd, s, h


_SLOT_OFF = []
_OUT_OFF = []
_off = 0
_ooff = 0
for _j in range(N_SLOTS):
    _NJ, _wd, _s, _h = _slot_geom(_j)
    _SLOT_OFF.append(_off)
    _OUT_OFF.append(_ooff)
    _off += _h * _s * _wd
    _ooff += B * _NJ
BLOB_ELEMS = _off
OUT_ELEMS = _ooff

# store groups: slots are emitted largest-first; group by position in that order
_ORDER = list(range(N_SLOTS - 1, -1, -1))
_GROUPS = [
    _ORDER[g * (N_SLOTS // N_STORE_GROUPS) : (g + 1) * (N_SLOTS // N_STORE_GROUPS)]
    for g in range(N_STORE_GROUPS)
]

_compiled_nc = None


def _build_program():
    global _compiled_nc
    if _compiled_nc is not None:
        return _compiled_nc

    from contextlib import ExitStack

    nc = bacc.Bacc("TRN2", target_bir_lowering=False, debug=False)
    f32 = mybir.dt.float32
    mm_dt = {"f32": f32, "f32r": mybir.dt.float32r}[KCFG["compute"]]
    blob = nc.dram_tensor("blob", [BLOB_ELEMS], f32, kind="ExternalInput").ap()
    outb = nc.dram_tensor("outblob", [OUT_ELEMS], f32, kind="ExternalOutput").ap()

    with tile.TileContext(nc) as tc, ExitStack() as ctx:
        in_pool = ctx.enter_context(tc.tile_pool(name="in", bufs=KCFG["in_bufs"]))
        acc_pool = ctx.enter_context(tc.tile_pool(name="acc", bufs=1))
        psum_pool = ctx.enter_context(
            tc.tile_pool(name="psum", bufs=KCFG["psum_bufs"], space="PSUM")
        )

        for g, slots in enumerate(_GROUPS):
            gw = sum(8 * (j + 1) for j in slots)  # total out columns of group
            acc_t = acc_pool.tile([B, gw], f32, tag=f"acc{g}")
            col = 0
            for j in slots:
                NJ, wd, s, h = _slot_geom(j)
                base = _SLOT_OFF[j]

                t = in_pool.tile([h, s * wd], f32)
                src = blob[base : base + h * s * wd].rearrange(
                    "(p f) -> p f", p=h, f=s * wd
                )
                nc.gpsimd.dma_start(t[:], src)

                psum_t = psum_pool.tile([B, NJ], f32)
                tv = t[:].bitcast(mm_dt)
                for c in range(s):
                    nc.tensor.matmul(
                        psum_t[:],
                        tv[:, c * wd : c * wd + DCOL],
                        tv[:, c * wd + DCOL : (c + 1) * wd],
                        start=(c == 0),
                        stop=(c == s - 1),
                    )
                nc.vector.tensor_copy(acc_t[:, col : col + NJ], psum_t[:])
                col += NJ
            # one store for the whole group; group slots are contiguous in
            # the out blob iff emitted in blob order — use per-slot offsets
            # via a single strided write per slot region is not possible,
            # so lay the out blob in GROUP order instead (host unpacks).
            dst = outb[_GOUT_OFF[g] : _GOUT_OFF[g] + B * gw].rearrange(
                "(p w) -> p w", p=B, w=gw
            )
            nc.gpsimd.dma_start(dst, acc_t[:])

    nc.compile()
    _compiled_nc = nc
    return nc


# out blob is laid out by store group, slots in _GROUPS order
_GOUT_OFF = []
_SLOT_OUT = {}  # j -> (group, col offset within group)
_goff = 0
for _g, _slots in enumerate(_GROUPS):
    _GOUT_OFF.append(_goff)
    _col = 0
    for _j in _slots:
        _SLOT_OUT[_j] = (_g, _col)
        _col += 8 * (_j + 1)
    _goff += B * _col
assert _goff == OUT_ELEMS


def _pack_core(k, x, W, bias):
    blob = np.zeros(BLOB_ELEMS, np.float32)
    for j in range(N_SLOTS):
        i = N_CORES * j + k
        ni = i + 1
        NJ, wd, s, h = _slot_geom(j)
        M = np.zeros((h * s, wd), np.float32)
        r = np.arange(ni)
        M[:ni, :DCOL] = x[:, r, i - r].T               # D^T[r, b]
        M[:ni, DCOL : DCOL + ni] = W[i, :ni, :ni].T    # V[r, q]
        # partition-major: row c*h+p -> partition p, segment c
        pm = M.reshape(s, h, wd).transpose(1, 0, 2)
        blob[_SLOT_OFF[j] : _SLOT_OFF[j] + h * s * wd] = pm.reshape(-1)
    return blob


def kernel(x, W, b):
    x = np.asarray(x, np.float32)
    W = np.asarray(W, np.float32)
    b = np.asarray(b, np.float32)

    nc = _build_program()
    in_maps = [{"blob": _pack_core(k, x, W, b)} for k in range(N_CORES)]
    res = run_bass_kernel_spmd(nc, in_maps, list(range(N_CORES)))

    y = x.copy()
    for k in range(N_CORES):
        ob = res.results[k]["outblob"]
        for j in range(N_SLOTS):
            i = N_CORES * j + k
            ni = i + 1
            NJ = 8 * (j + 1)
            g, col = _SLOT_OUT[j]
            gw = sum(8 * (jj + 1) for jj in _GROUPS[g])
            og = ob[_GOUT_OFF[g] : _GOUT_OFF[g] + B * gw].reshape(B, gw)
            q = np.arange(ni)
            y[:, q, i - q] = og[:, col : col + ni] + b[i, :ni][None]
    return y


# revision 7
# speedup vs baseline: 2.3493x; 1.0871x over previous
"""Trainium2 Bass kernel for nn_DiagonalTraining (ragged per-anti-diagonal linear).

Math (reference): for each batch image x[b] (SxS) and each anti-diagonal
i (elements x[b, r, i-r], r=0..i), apply a per-diagonal linear layer:
  out[b,i,q] = sum_{r<=i} x[b,r,i-r] * W[i,q,r] + bias[i,q]   (q <= i)
and scatter back: y[b,q,i-q] = out[b,i,q]; positions with r+c >= S keep x.

Distribution: diagonal i -> core i%8, slot j=i//8 (64 slots per core,
balanced by construction). Host packs, per (core, slot), an augmented
matrix whose rows are the contraction axis r:
  [ D^T | V ]  with D^T[r,b]=x[b,r,i-r], V[r,q]=W[i,q,r]  (r,q < ni=i+1)
zero-padded to a core-independent size NJ=8*(j+1) (>= ni for every
core) so the SPMD program is identical on all cores. The per-diagonal
bias is added on the host while scattering results back (elementwise,
~0.05% of the FLOPs; the whole einsum runs on device).

Device: per slot, ONE SWDGE DMA loads the slot in partition-major
layout (row c*h+p stored at partition p, segment c), giving h (<=128)
large descriptors balanced across all 16 SDMA engines. The tensor
engine accumulates  psum[32, NJ] += chunk_c[:, :32].T @ chunk_c[:, 32:]
over the s chunks (float32r operands: full 32-bit data, 1 cycle/column
at N>=256), DVE copies psum into a quarter-group SBUF accumulator, and
one DMA per quarter stores the packed outputs. Host scatters them back
into a copy of x.

Only the live (lower-triangular) part of W is shipped/read (~25 MB/core
vs 512 MB full W) — the kernel is HBM-bound on exactly those bytes.
"""

import sys

for _p in ("/opt/trn_rl_repo", "/opt/pypackages"):
    if _p not in sys.path:
        sys.path.append(_p)

import numpy as np

import concourse.bass as bass  # noqa: F401
import concourse.tile as tile
from concourse import bacc, mybir
from concourse.bass_utils import run_bass_kernel_spmd

B = 32          # batch
S = 512         # seq len / number of diagonals
N_CORES = 8
N_SLOTS = S // N_CORES  # 64 slots per core
DCOL = B        # width of the D^T block (batch on matmul M axis)
N_STORE_GROUPS = 4

# tuning knobs (test.py may override before first kernel() call)
KCFG = {
    "compute": "f32r",  # "f32" | "f32r"
    "in_bufs": 4,
    "psum_bufs": 8,
}


def _slot_geom(j):
    """(NJ, wd, s, h): chunk count s, chunk height h (=partitions)."""
    NJ = 8 * (j + 1)
    wd = DCOL + NJ
    s = -(-NJ // 128)                   # ceil
    h = -(-NJ // s)
    return NJ, wd, s, h


_SLOT_OFF = []
_OUT_OFF = []
_off = 0
_ooff = 0
for _j in range(N_SLOTS):
    _NJ, _wd, _s, _h = _slot_geom(_j)
    _SLOT_OFF.append(_off)
    _OUT_OFF.append(_ooff)
    _off += _h * _s * _wd
    _ooff += B * _NJ
BLOB_ELEMS = _off
OUT_ELEMS = _ooff

# store groups: slots are emitted largest-first; group by position in that order
_ORDER = list(range(N_SLOTS - 1, -1, -1))
_GROUPS = [
    _ORDER[g * (N_SLOTS // N_STORE_GROUPS) : (g + 1) * (N_SLOTS // N_STORE_GROUPS)]
    for g in range(N_STORE_GROUPS)
]

_compiled_nc = None


def _build_program():
    global _compiled_nc
    if _compiled_nc is not None:
        return _compiled_nc

    from contextlib import ExitStack

    nc = bacc.Bacc("TRN2", target_bir_lowering=False, debug=False)
    f32 = mybir.dt.float32
    mm_dt = {"f32": f32, "f32r": mybir.dt.float32r}[KCFG["compute"]]
    blob = nc.dram_tensor("blob", [BLOB_ELEMS], mm_dt, kind="ExternalInput").ap()
    outb = nc.dram_tensor("outblob", [OUT_ELEMS], f32, kind="ExternalOutput").ap()

    with tile.TileContext(nc) as tc, ExitStack() as ctx:
        in_pool = ctx.enter_context(tc.tile_pool(name="in", bufs=KCFG["in_bufs"]))
        acc_pool = ctx.enter_context(tc.tile_pool(name="acc", bufs=1))
        psum_pool = ctx.enter_context(
            tc.tile_pool(name="psum", bufs=KCFG["psum_bufs"], space="PSUM")
        )

        for g, slots in enumerate(_GROUPS):
            gw = sum(8 * (j + 1) for j in slots)  # total out columns of group
            acc_t = acc_pool.tile([B, gw], f32, tag=f"acc{g}")
            col = 0
            for j in slots:
                NJ, wd, s, h = _slot_geom(j)
                base = _SLOT_OFF[j]

                t = in_pool.tile([h, s * wd], mm_dt)
                src = blob[base : base + h * s * wd].rearrange(
                    "(p f) -> p f", p=h, f=s * wd
                )
                nc.gpsimd.dma_start(t[:], src)

                psum_t = psum_pool.tile([B, NJ], f32)
                tv = t[:]
                for c in range(s):
                    nc.tensor.matmul(
                        psum_t[:],
                        tv[:, c * wd : c * wd + DCOL],
                        tv[:, c * wd + DCOL : (c + 1) * wd],
                        start=(c == 0),
                        stop=(c == s - 1),
                    )
                nc.vector.tensor_copy(acc_t[:, col : col + NJ], psum_t[:])
                col += NJ
            # one store for the whole group; group slots are contiguous in
            # the out blob iff emitted in blob order — use per-slot offsets
            # via a single strided write per slot region is not possible,
            # so lay the out blob in GROUP order instead (host unpacks).
            dst = outb[_GOUT_OFF[g] : _GOUT_OFF[g] + B * gw].rearrange(
                "(p w) -> p w", p=B, w=gw
            )
            nc.gpsimd.dma_start(dst, acc_t[:])

    nc.compile()
    _compiled_nc = nc
    return nc


# out blob is laid out by store group, slots in _GROUPS order
_GOUT_OFF = []
_SLOT_OUT = {}  # j -> (group, col offset within group)
_goff = 0
for _g, _slots in enumerate(_GROUPS):
    _GOUT_OFF.append(_goff)
    _col = 0
    for _j in _slots:
        _SLOT_OUT[_j] = (_g, _col)
        _col += 8 * (_j + 1)
    _goff += B * _col
assert _goff == OUT_ELEMS


def _pack_core(k, x, W, bias):
    blob = np.zeros(BLOB_ELEMS, np.float32)
    for j in range(N_SLOTS):
        i = N_CORES * j + k
        ni = i + 1
        NJ, wd, s, h = _slot_geom(j)
        M = np.zeros((h * s, wd), np.float32)
        r = np.arange(ni)
        M[:ni, :DCOL] = x[:, r, i - r].T               # D^T[r, b]
        M[:ni, DCOL : DCOL + ni] = W[i, :ni, :ni].T    # V[r, q]
        # partition-major: row c*h+p -> partition p, segment c
        pm = M.reshape(s, h, wd).transpose(1, 0, 2)
        blob[_SLOT_OFF[j] : _SLOT_OFF[j] + h * s * wd] = pm.reshape(-1)
    return blob


def kernel(x, W, b):
    x = np.asarray(x, np.float32)
    W = np.asarray(W, np.float32)
    b = np.asarray(b, np.float32)

    nc = _build_program()
    in_maps = [{"blob": _pack_core(k, x, W, b)} for k in range(N_CORES)]
    res = run_bass_kernel_spmd(nc, in_maps, list(range(N_CORES)))

    y = x.copy()
    for k in range(N_CORES):
        ob = res.results[k]["outblob"]
        for j in range(N_SLOTS):
            i = N_CORES * j + k
            ni = i + 1
            NJ = 8 * (j + 1)
            g, col = _SLOT_OUT[j]
            gw = sum(8 * (jj + 1) for jj in _GROUPS[g])
            og = ob[_GOUT_OFF[g] : _GOUT_OFF[g] + B * gw].reshape(B, gw)
            q = np.arange(ni)
            y[:, q, i - q] = og[:, col : col + ni] + b[i, :ni][None]
    return y


# revision 9
# speedup vs baseline: 2.6050x; 1.1089x over previous
"""Trainium2 Bass kernel for nn_DiagonalTraining (ragged per-anti-diagonal linear).

Math (reference): for each batch image x[b] (SxS) and each anti-diagonal
i (elements x[b, r, i-r], r=0..i), apply a per-diagonal linear layer:
  out[b,i,q] = sum_{r<=i} x[b,r,i-r] * W[i,q,r] + bias[i,q]   (q <= i)
and scatter back: y[b,q,i-q] = out[b,i,q]; positions with r+c >= S keep x.

Distribution: diagonal i -> core i%8, slot j=i//8 (64 slots per core,
balanced by construction). Host packs, per (core, slot), an augmented
matrix whose rows are the contraction axis r:
  [ D^T | V ]  with D^T[r,b]=x[b,r,i-r], V[r,q]=W[i,q,r]  (r,q < ni=i+1)
zero-padded to a core-independent size NJ=8*(j+1) (>= ni for every
core) so the SPMD program is identical on all cores. The per-diagonal
bias is added on the host while scattering results back (elementwise,
~0.05% of the FLOPs; the whole einsum runs on device).

Device: per slot, ONE SWDGE DMA loads the slot in partition-major
layout (row c*h+p stored at partition p, segment c), giving h (<=128)
large descriptors balanced across all 16 SDMA engines. The tensor
engine accumulates  psum[32, NJ] += chunk_c[:, :32].T @ chunk_c[:, 32:]
over the s chunks (float32r operands: full 32-bit data, 1 cycle/column
at N>=256), DVE copies psum into a quarter-group SBUF accumulator, and
one DMA per quarter stores the packed outputs. Host scatters them back
into a copy of x.

Only the live (lower-triangular) part of W is shipped/read (~25 MB/core
vs 512 MB full W) — the kernel is HBM-bound on exactly those bytes.
"""

import sys

for _p in ("/opt/trn_rl_repo", "/opt/pypackages"):
    if _p not in sys.path:
        sys.path.append(_p)

import numpy as np

import concourse.bass as bass  # noqa: F401
import concourse.tile as tile
from concourse import bacc, mybir
from concourse.bass_utils import run_bass_kernel_spmd

B = 32          # batch
S = 512         # seq len / number of diagonals
N_CORES = 8
N_SLOTS = S // N_CORES  # 64 slots per core
DCOL = B        # width of the D^T block (batch on matmul M axis)
N_STORE_GROUPS = 4

# tuning knobs (test.py may override before first kernel() call)
KCFG = {
    "compute": "f32r",  # "f32" | "f32r"
    "in_bufs": 6,
    "psum_bufs": 8,
}


def _slot_geom(j):
    """(NJ, wd, s, h): chunk count s, chunk height h (=partitions)."""
    NJ = 8 * (j + 1)
    wd = DCOL + NJ
    s = -(-NJ // 128)                   # ceil
    h = -(-NJ // s)
    return NJ, wd, s, h


_SLOT_OFF = []
_OUT_OFF = []
_off = 0
_ooff = 0
for _j in range(N_SLOTS):
    _NJ, _wd, _s, _h = _slot_geom(_j)
    _SLOT_OFF.append(_off)
    _OUT_OFF.append(_ooff)
    _off += _h * _s * _wd
    _ooff += B * _NJ
BLOB_ELEMS = _off
OUT_ELEMS = _ooff

# store groups: slots are emitted largest-first; group by position in that order
_ORDER = list(range(N_SLOTS - 1, -1, -1))
_GROUPS = [
    _ORDER[g * (N_SLOTS // N_STORE_GROUPS) : (g + 1) * (N_SLOTS // N_STORE_GROUPS)]
    for g in range(N_STORE_GROUPS)
]

_compiled_nc = None


def _build_program():
    global _compiled_nc
    if _compiled_nc is not None:
        return _compiled_nc

    from contextlib import ExitStack

    nc = bacc.Bacc("TRN2", target_bir_lowering=False, debug=False)
    f32 = mybir.dt.float32
    mm_dt = {"f32": f32, "f32r": mybir.dt.float32r}[KCFG["compute"]]
    blob = nc.dram_tensor("blob", [BLOB_ELEMS], mm_dt, kind="ExternalInput").ap()
    outb = nc.dram_tensor("outblob", [OUT_ELEMS], f32, kind="ExternalOutput").ap()

    with tile.TileContext(nc) as tc, ExitStack() as ctx:
        in_pool = ctx.enter_context(tc.tile_pool(name="in", bufs=KCFG["in_bufs"]))
        acc_pool = ctx.enter_context(tc.tile_pool(name="acc", bufs=1))
        psum_pool = ctx.enter_context(
            tc.tile_pool(name="psum", bufs=KCFG["psum_bufs"], space="PSUM")
        )

        acc_tiles = []
        for g, slots in enumerate(_GROUPS):
            gw = sum(8 * (j + 1) for j in slots)  # total out columns of group
            acc_t = acc_pool.tile([B, gw], f32, tag=f"acc{g}")
            acc_tiles.append((g, gw, acc_t))
            col = 0
            for j in slots:
                NJ, wd, s, h = _slot_geom(j)
                base = _SLOT_OFF[j]

                t = in_pool.tile([h, s * wd], mm_dt)
                src = blob[base : base + h * s * wd].rearrange(
                    "(p f) -> p f", p=h, f=s * wd
                )
                nc.gpsimd.dma_start(t[:], src)

                psum_t = psum_pool.tile([B, NJ], f32)
                tv = t[:]
                for c in range(s):
                    nc.tensor.matmul(
                        psum_t[:],
                        tv[:, c * wd : c * wd + DCOL],
                        tv[:, c * wd + DCOL : (c + 1) * wd],
                        start=(c == 0),
                        stop=(c == s - 1),
                    )
                nc.vector.tensor_copy(acc_t[:, col : col + NJ], psum_t[:])
                col += NJ
        # stores LAST on gpsimd: a mid-stream store would sit in the Pool
        # sequencer waiting for its group's copies and block later loads.
        for g, gw, acc_t in acc_tiles:
            dst = outb[_GOUT_OFF[g] : _GOUT_OFF[g] + B * gw].rearrange(
                "(p w) -> p w", p=B, w=gw
            )
            nc.gpsimd.dma_start(dst, acc_t[:])

    nc.compile()
    _compiled_nc = nc
    return nc


# out blob is laid out by store group, slots in _GROUPS order
_GOUT_OFF = []
_SLOT_OUT = {}  # j -> (group, col offset within group)
_goff = 0
for _g, _slots in enumerate(_GROUPS):
    _GOUT_OFF.append(_goff)
    _col = 0
    for _j in _slots:
        _SLOT_OUT[_j] = (_g, _col)
        _col += 8 * (_j + 1)
    _goff += B * _col
assert _goff == OUT_ELEMS


def _pack_core(k, x, W, bias):
    blob = np.zeros(BLOB_ELEMS, np.float32)
    for j in range(N_SLOTS):
        i = N_CORES * j + k
        ni = i + 1
        NJ, wd, s, h = _slot_geom(j)
        M = np.zeros((h * s, wd), np.float32)
        r = np.arange(ni)
        M[:ni, :DCOL] = x[:, r, i - r].T               # D^T[r, b]
        M[:ni, DCOL : DCOL + ni] = W[i, :ni, :ni].T    # V[r, q]
        # partition-major: row c*h+p -> partition p, segment c
        pm = M.reshape(s, h, wd).transpose(1, 0, 2)
        blob[_SLOT_OFF[j] : _SLOT_OFF[j] + h * s * wd] = pm.reshape(-1)
    return blob


def kernel(x, W, b):
    x = np.asarray(x, np.float32)
    W = np.asarray(W, np.float32)
    b = np.asarray(b, np.float32)

    nc = _build_program()
    in_maps = [{"blob": _pack_core(k, x, W, b)} for k in range(N_CORES)]
    res = run_bass_kernel_spmd(nc, in_maps, list(range(N_CORES)))

    y = x.copy()
    for k in range(N_CORES):
        ob = res.results[k]["outblob"]
        for j in range(N_SLOTS):
            i = N_CORES * j + k
            ni = i + 1
            NJ = 8 * (j + 1)
            g, col = _SLOT_OUT[j]
            gw = sum(8 * (jj + 1) for jj in _GROUPS[g])
            og = ob[_GOUT_OFF[g] : _GOUT_OFF[g] + B * gw].reshape(B, gw)
            q = np.arange(ni)
            y[:, q, i - q] = og[:, col : col + ni] + b[i, :ni][None]
    return y


# revision 15
# speedup vs baseline: 2.7334x; 1.0493x over previous
"""Trainium2 Bass kernel for nn_DiagonalTraining (ragged per-anti-diagonal linear).

Math (reference): for each batch image x[b] (SxS) and each anti-diagonal
i (elements x[b, r, i-r], r=0..i), apply a per-diagonal linear layer:
  out[b,i,q] = sum_{r<=i} x[b,r,i-r] * W[i,q,r] + bias[i,q]   (q <= i)
and scatter back: y[b,q,i-q] = out[b,i,q]; positions with r+c >= S keep x.

Distribution: diagonal i -> core i%8, slot j=i//8 (64 slots per core,
balanced by construction). Host packs, per (core, slot), an augmented
matrix whose rows are the contraction axis r:
  [ D^T | V ]  with D^T[r,b]=x[b,r,i-r], V[r,q]=W[i,q,r]  (r,q < ni=i+1)
zero-padded to a core-independent size NJ=8*(j+1) (>= ni for every
core) so the SPMD program is identical on all cores. The per-diagonal
bias is added on the host while scattering results back (elementwise,
~0.05% of the FLOPs; the whole einsum runs on device).

Device: per slot, ONE SWDGE DMA loads the slot in partition-major
layout (row c*h+p stored at partition p, segment c), giving h (<=128)
large descriptors balanced across all 16 SDMA engines. The tensor
engine accumulates  psum[32, NJ] += chunk_c[:, :32].T @ chunk_c[:, 32:]
over the s chunks (float32r operands: full 32-bit data, 1 cycle/column
at N>=256), DVE copies psum into a quarter-group SBUF accumulator, and
one DMA per quarter stores the packed outputs. Host scatters them back
into a copy of x.

Only the live (lower-triangular) part of W is shipped/read (~25 MB/core
vs 512 MB full W) — the kernel is HBM-bound on exactly those bytes.
"""

import sys

for _p in ("/opt/trn_rl_repo", "/opt/pypackages"):
    if _p not in sys.path:
        sys.path.append(_p)

import numpy as np

import concourse.bass as bass  # noqa: F401
import concourse.tile as tile
from concourse import bacc, mybir
from concourse.bass_utils import run_bass_kernel_spmd

B = 32          # batch
S = 512         # seq len / number of diagonals
N_CORES = 8
N_SLOTS = S // N_CORES  # 64 slots per core
DCOL = B        # width of the D^T block (batch on matmul M axis)
N_STORE_GROUPS = 16  # 4 slots per group -> psum group tile <= 4 PSUM banks

# tuning knobs (test.py may override before first kernel() call)
KCFG = {
    "compute": "f32r",  # "f32" | "f32r"
    "in_bufs": 10,
    "psum_bufs": 2,     # group psum tiles (4 banks each)
}


def _slot_geom(j):
    """(NJ, wd, s, h): chunk count s, chunk height h (=partitions)."""
    NJ = 8 * (j + 1)
    wd = DCOL + NJ
    s = -(-NJ // 128)                   # ceil
    h = -(-NJ // s)
    return NJ, wd, s, h


_SLOT_OFF = []
_OUT_OFF = []
_off = 0
_ooff = 0
for _j in range(N_SLOTS):
    _NJ, _wd, _s, _h = _slot_geom(_j)
    _SLOT_OFF.append(_off)
    _OUT_OFF.append(_ooff)
    _off += _h * _s * _wd
    _ooff += B * _NJ
BLOB_ELEMS = _off
OUT_ELEMS = _ooff

# store groups: slots are emitted largest-first; group by position in that order
_ORDER = list(range(N_SLOTS - 1, -1, -1))
_GROUPS = [
    _ORDER[g * (N_SLOTS // N_STORE_GROUPS) : (g + 1) * (N_SLOTS // N_STORE_GROUPS)]
    for g in range(N_STORE_GROUPS)
]

# Column layout within each group's psum/staging tile. A single matmul
# output [B, NJ] must NOT straddle a PSUM bank boundary (512 f32), so
# bump a slot's column offset to the next bank edge when it would.
_BANK = 512
_GROUP_COLS = []   # per group: list of (j, col)
_GROUP_W = []      # per group: padded width
for _slots in _GROUPS:
    _col = 0
    _cols = []
    for _j in _slots:
        _NJ = 8 * (_j + 1)
        if _col // _BANK != (_col + _NJ - 1) // _BANK:
            _col = ((_col + _BANK - 1) // _BANK) * _BANK
        _cols.append((_j, _col))
        _col += _NJ
    _GROUP_COLS.append(_cols)
    _GROUP_W.append(_col)

_compiled_nc = None


def _build_program():
    global _compiled_nc
    if _compiled_nc is not None:
        return _compiled_nc

    from contextlib import ExitStack

    nc = bacc.Bacc("TRN2", target_bir_lowering=False, debug=False)
    f32 = mybir.dt.float32
    mm_dt = {"f32": f32, "f32r": mybir.dt.float32r}[KCFG["compute"]]
    blob = nc.dram_tensor("blob", [BLOB_ELEMS], mm_dt, kind="ExternalInput").ap()
    outb = nc.dram_tensor("outblob", [OUT_ELEMS], f32, kind="ExternalOutput").ap()

    with tile.TileContext(nc) as tc, ExitStack() as ctx:
        in_pool = ctx.enter_context(tc.tile_pool(name="in", bufs=KCFG["in_bufs"]))
        acc_pool = ctx.enter_context(tc.tile_pool(name="acc", bufs=1))
        psum_pool = ctx.enter_context(
            tc.tile_pool(name="psum", bufs=KCFG["psum_bufs"], space="PSUM")
        )

        acc_tiles = []
        for g, slots in enumerate(_GROUPS):
            gw = _GROUP_W[g]
            acc_t = acc_pool.tile([B, gw], f32, tag=f"acc{g}")
            acc_tiles.append((g, gw, acc_t))
            # one psum tile per group (<= 4 PSUM banks); each slot's matmuls
            # accumulate into its own bank-aligned column range
            psum_t = psum_pool.tile([B, gw], f32)
            for j, col in _GROUP_COLS[g]:
                NJ, wd, s, h = _slot_geom(j)
                base = _SLOT_OFF[j]

                t = in_pool.tile([h, s * wd], mm_dt)
                src = blob[base : base + h * s * wd].rearrange(
                    "(p f) -> p f", p=h, f=s * wd
                )
                nc.gpsimd.dma_start(t[:], src)

                tv = t[:]
                for c in range(s):
                    nc.tensor.matmul(
                        psum_t[:, col : col + NJ],
                        tv[:, c * wd : c * wd + DCOL],
                        tv[:, c * wd + DCOL : (c + 1) * wd],
                        start=(c == 0),
                        stop=(c == s - 1),
                    )
            nc.vector.tensor_copy(acc_t[:], psum_t[:])
        # stores LAST on gpsimd so no store wait ever parks in front of a
        # load in the Pool sequencer stream.
        for g, gw, acc_t in acc_tiles:
            dst = outb[_GOUT_OFF[g] : _GOUT_OFF[g] + B * gw].rearrange(
                "(p w) -> p w", p=B, w=gw
            )
            nc.gpsimd.dma_start(dst, acc_t[:])

    nc.compile()
    _compiled_nc = nc
    return nc


# out blob is laid out by store group (bank-padded columns)
_GOUT_OFF = []
_SLOT_OUT = {}  # j -> (group, col offset within group)
_goff = 0
for _g in range(N_STORE_GROUPS):
    _GOUT_OFF.append(_goff)
    for _j, _col in _GROUP_COLS[_g]:
        _SLOT_OUT[_j] = (_g, _col)
    _goff += B * _GROUP_W[_g]
OUT_ELEMS = _goff


def _pack_core(k, x, W, bias):
    blob = np.zeros(BLOB_ELEMS, np.float32)
    for j in range(N_SLOTS):
        i = N_CORES * j + k
        ni = i + 1
        NJ, wd, s, h = _slot_geom(j)
        M = np.zeros((h * s, wd), np.float32)
        r = np.arange(ni)
        M[:ni, :DCOL] = x[:, r, i - r].T               # D^T[r, b]
        M[:ni, DCOL : DCOL + ni] = W[i, :ni, :ni].T    # V[r, q]
        # partition-major: row c*h+p -> partition p, segment c
        pm = M.reshape(s, h, wd).transpose(1, 0, 2)
        blob[_SLOT_OFF[j] : _SLOT_OFF[j] + h * s * wd] = pm.reshape(-1)
    return blob


def kernel(x, W, b):
    x = np.asarray(x, np.float32)
    W = np.asarray(W, np.float32)
    b = np.asarray(b, np.float32)

    nc = _build_program()
    in_maps = [{"blob": _pack_core(k, x, W, b)} for k in range(N_CORES)]
    res = run_bass_kernel_spmd(nc, in_maps, list(range(N_CORES)))

    y = x.copy()
    for k in range(N_CORES):
        ob = res.results[k]["outblob"]
        for j in range(N_SLOTS):
            i = N_CORES * j + k
            ni = i + 1
            NJ = 8 * (j + 1)
            g, col = _SLOT_OUT[j]
            gw = _GROUP_W[g]
            og = ob[_GOUT_OFF[g] : _GOUT_OFF[g] + B * gw].reshape(B, gw)
            q = np.arange(ni)
            y[:, q, i - q] = og[:, col : col + ni] + b[i, :ni][None]
    return y


# revision 17
# speedup vs baseline: 5.4199x; 1.9829x over previous
"""Trainium2 Bass kernel for nn_DiagonalTraining (ragged per-anti-diagonal linear).

Math (reference): for each batch image x[b] (SxS) and each anti-diagonal
i (elements x[b, r, i-r], r=0..i), apply a per-diagonal linear layer:
  out[b,i,q] = sum_{r<=i} x[b,r,i-r] * W[i,q,r] + bias[i,q]   (q <= i)
and scatter back: y[b,q,i-q] = out[b,i,q]; positions with r+c >= S keep x.

Distribution: diagonal i -> core i%8, slot j=i//8 (64 slots per core,
balanced by construction). Host packs, per (core, slot), an augmented
matrix whose rows are the contraction axis r:
  [ D^T | V ]  with D^T[r,b]=x[b,r,i-r], V[r,q]=W[i,q,r]  (r,q < ni=i+1)
zero-padded to a core-independent size NJ=8*(j+1) (>= ni for every
core) so the SPMD program is identical on all cores. The per-diagonal
bias is added on the host while scattering results back (elementwise,
~0.05% of the FLOPs; the whole einsum runs on device).

Device ("window streaming"): each slot is split into row-chunks padded
to 128 rows; chunk columns ([128, 32+NJ] blocks) are packed first-fit
into uniform [128, WF] window tiles. The windows are loaded by ~18
identical big SWDGE DMAs (128 descriptors of WF*4 bytes each) — full
128-partition DMAs spread evenly over all 16 SDMA engines and stream
at near-HBM rate, fully decoupled from compute. Matmuls read chunks at
static (window, column) offsets, accumulating psum[32, NJ] per slot
inside a bank-packed 4-slot group psum tile; one DVE copy per group
stages results, and all group stores run at the end of the SWDGE queue.

Only the live (lower-triangular) part of W is shipped/read (~29 MB/core
vs 512 MB full W) — the kernel is HBM-bound on ~those bytes.
float32r matmul operands: full 32-bit data, 1 cycle/column at N>=256.
"""

import sys

for _p in ("/opt/trn_rl_repo", "/opt/pypackages"):
    if _p not in sys.path:
        sys.path.append(_p)

import numpy as np

import concourse.bass as bass  # noqa: F401
import concourse.tile as tile
from concourse import bacc, mybir
from concourse.bass_utils import run_bass_kernel_spmd

B = 32          # batch
S = 512         # seq len / number of diagonals
N_CORES = 8
N_SLOTS = S // N_CORES  # 64 slots per core
DCOL = B        # width of the D^T block (batch on matmul M axis)
GROUP = 4       # slots per psum group
N_GROUPS = N_SLOTS // GROUP
WF = 3072       # window free size (f32 elems per partition) = 12 KiB descs

KCFG = {
    "compute": "f32r",  # "f32" | "f32r" | "bf16"
    "win_bufs": 8,
    "psum_bufs": 2,
}

# ---- static layout ----------------------------------------------------
# processing order: largest slot first
_ORDER = list(range(N_SLOTS - 1, -1, -1))
_GROUPS = [_ORDER[g * GROUP : (g + 1) * GROUP] for g in range(N_GROUPS)]

# chunks per slot: rows padded to 128; chunk = [128, wd] column block
# placement: first-fit in order into [128, WF] windows
_SLOT_CHUNKS = {}   # j -> list of (win, cbase, rows_live)
_cur_win = 0
_cur_col = 0
for _j in _ORDER:
    _NJ = 8 * (_j + 1)
    _wd = DCOL + _NJ
    _s = -(-_NJ // 128)
    _chs = []
    for _c in range(_s):
        _rows = min(128, _NJ - 128 * _c)
        if _cur_col + _wd > WF:
            _cur_win += 1
            _cur_col = 0
        _chs.append((_cur_win, _cur_col, _rows))
        _cur_col += _wd
    _SLOT_CHUNKS[_j] = _chs
N_WINS = _cur_win + 1
_LAST_COL = _cur_col
BLOB_ELEMS = 128 * (WF * (N_WINS - 1) + _LAST_COL)

# last window index each slot needs (for bookkeeping/debug)
_SLOT_LASTWIN = {j: max(w for w, _, _ in chs) for j, chs in _SLOT_CHUNKS.items()}

# psum group column layout (bank-aligned, no matmul straddles a bank)
_BANK = 512
_GROUP_COLS = []
_GROUP_W = []
for _slots in _GROUPS:
    _col = 0
    _cols = []
    for _j in _slots:
        _NJ = 8 * (_j + 1)
        if _col // _BANK != (_col + _NJ - 1) // _BANK:
            _col = ((_col + _BANK - 1) // _BANK) * _BANK
        _cols.append((_j, _col))
        _col += _NJ
    _GROUP_COLS.append(_cols)
    _GROUP_W.append(_col)

_GOUT_OFF = []
_SLOT_OUT = {}
_goff = 0
for _g in range(N_GROUPS):
    _GOUT_OFF.append(_goff)
    for _j, _col in _GROUP_COLS[_g]:
        _SLOT_OUT[_j] = (_g, _col)
    _goff += B * _GROUP_W[_g]
OUT_ELEMS = _goff

_compiled_nc = None


def _build_program():
    global _compiled_nc
    if _compiled_nc is not None:
        return _compiled_nc

    from contextlib import ExitStack

    nc = bacc.Bacc("TRN2", target_bir_lowering=False, debug=False)
    f32 = mybir.dt.float32
    mm_dt = {
        "f32": f32,
        "f32r": mybir.dt.float32r,
        "bf16": mybir.dt.bfloat16,
    }[KCFG["compute"]]
    blob = nc.dram_tensor("blob", [BLOB_ELEMS], mm_dt, kind="ExternalInput").ap()
    outb = nc.dram_tensor("outblob", [OUT_ELEMS], f32, kind="ExternalOutput").ap()

    with tile.TileContext(nc) as tc, ExitStack() as ctx:
        win_pool = ctx.enter_context(
            tc.tile_pool(name="win", bufs=KCFG["win_bufs"])
        )
        acc_pool = ctx.enter_context(tc.tile_pool(name="acc", bufs=1))
        psum_pool = ctx.enter_context(
            tc.tile_pool(name="psum", bufs=KCFG["psum_bufs"], space="PSUM")
        )

        # window tiles are loaded lazily in program order; keep handles
        win_tiles = [None] * N_WINS

        def ensure_win(w):
            if win_tiles[w] is None:
                wf = WF if w < N_WINS - 1 else _LAST_COL
                t = win_pool.tile([128, wf], mm_dt)
                src = blob[128 * WF * w : 128 * WF * w + 128 * wf].rearrange(
                    "(p f) -> p f", p=128, f=wf
                )
                nc.gpsimd.dma_start(t[:], src)
                win_tiles[w] = t
            return win_tiles[w]

        acc_tiles = []
        for g, slots in enumerate(_GROUPS):
            gw = _GROUP_W[g]
            acc_t = acc_pool.tile([B, gw], f32, tag=f"acc{g}")
            acc_tiles.append((g, gw, acc_t))
            psum_t = psum_pool.tile([B, gw], f32)
            for j, col in _GROUP_COLS[g]:
                NJ = 8 * (j + 1)
                wd = DCOL + NJ
                chs = _SLOT_CHUNKS[j]
                for c, (w, cb, rows) in enumerate(chs):
                    t = ensure_win(w)
                    nc.tensor.matmul(
                        psum_t[:, col : col + NJ],
                        t[0:rows, cb : cb + DCOL],
                        t[0:rows, cb + DCOL : cb + wd],
                        start=(c == 0),
                        stop=(c == len(chs) - 1),
                    )
            nc.vector.tensor_copy(acc_t[:], psum_t[:])
        # stores LAST on gpsimd so no store wait ever parks in front of a
        # load in the Pool sequencer stream.
        for g, gw, acc_t in acc_tiles:
            dst = outb[_GOUT_OFF[g] : _GOUT_OFF[g] + B * gw].rearrange(
                "(p w) -> p w", p=B, w=gw
            )
            nc.gpsimd.dma_start(dst, acc_t[:])

    nc.compile()
    _compiled_nc = nc
    return nc


def _pack_core(k, x, W, bias):
    np_dt = np.float32
    if KCFG["compute"] == "bf16":
        import ml_dtypes

        np_dt = ml_dtypes.bfloat16
    blob = np.zeros(BLOB_ELEMS, np_dt)
    bw = blob.reshape(128, -1) if False else None  # noqa
    # windows: [128, WF] images laid out window-major
    for j in range(N_SLOTS):
        i = N_CORES * j + k
        ni = i + 1
        NJ = 8 * (j + 1)
        wd = DCOL + NJ
        M = np.zeros((NJ, wd), np.float32)
        r = np.arange(ni)
        M[:ni, :DCOL] = x[:, r, i - r].T               # D^T[r, b]
        M[:ni, DCOL : DCOL + ni] = W[i, :ni, :ni].T    # V[r, q]
        for c, (w, cb, rows) in enumerate(_SLOT_CHUNKS[j]):
            rl = M[128 * c : 128 * c + rows]           # [rows, wd]
            woff = 128 * WF * w
            wf = WF if w < N_WINS - 1 else _LAST_COL
            img = blob[woff : woff + 128 * wf].reshape(128, wf)
            img[:rows, cb : cb + wd] = rl.astype(np_dt)
    return blob


def kernel(x, W, b):
    x = np.asarray(x, np.float32)
    W = np.asarray(W, np.float32)
    b = np.asarray(b, np.float32)

    nc = _build_program()
    in_maps = [{"blob": _pack_core(k, x, W, b)} for k in range(N_CORES)]
    res = run_bass_kernel_spmd(nc, in_maps, list(range(N_CORES)))

    y = x.copy()
    for k in range(N_CORES):
        ob = res.results[k]["outblob"]
        for j in range(N_SLOTS):
            i = N_CORES * j + k
            ni = i + 1
            g, col = _SLOT_OUT[j]
            gw = _GROUP_W[g]
            og = ob[_GOUT_OFF[g] : _GOUT_OFF[g] + B * gw].reshape(B, gw)
            q = np.arange(ni)
            y[:, q, i - q] = og[:, col : col + ni] + b[i, :ni][None]
    return y
